# revision 47
# baseline (speedup 1.0000x reference)
"""Trainium2 Bass kernel for nn_Mlp_13099650253522 (BitNet-ternary dense MLP).

  h = gelu(x @ ter_quant(w1).T + b1);  y = h @ ter_quant(w2).T + b2
  ter_quant(w) = clip(round(w / g), -1, 1) * g,  g = mean(|w|) + 1e-5

v3.8 (8 NeuronCores, data-parallel over the 64*197=12608 tokens),
~230us vs 249.5us baseline:
 - w1 shipped as fp16 (host cast; ~0.007 rel err vs 2e-2 gate, verified
   offline) in a per-slice partition-contiguous layout; 14 chunk DMAs
   (slice5 tail split finest).  w1 critical DMA 9.4MB -> 4.7MB.
 - gamma1 |w| chunk-reduces alternate DVE / ACT(Abs+accum) so the
   reduction chases the DMA stream instead of serializing on DVE.
 - PE warm-up: junk matmuls gated on each chunk's reduce keep the HAM
   clock at 2.4GHz through the DMA phase (plus a bridge batch gated on
   the gamma chain).
 - Ternary quant, two paths:
     DVE2: b=(w<=-g/2); t=(w>=g/2)-b  -> {-1,0,1}, scale gamma.
     ACT-assist (slice0 cols 256:512 + slice1): ACT Sign(w-g/2) and
     Sign(w+g/2), DVE adds -> {-2,0,2}, gelu scale gamma/2.  This
     offloads DVE exactly where fc1's consumption outpaces it.
   Slice0 cols 0:256 in kd-triple column-quarter jobs (hc0/hc1 chase);
   slices 2-5 in [128,3,512] half-slice jobs.
 - w2 fp32, issued b5..b0 reusing w1 slots 0..5 (slot-availability
   order == issue order); fc2's kh loop consumes batches in LANDING
   order (b5 first) so t2 quant chases.  |w2| sums: b5..b1 on ACT
   half-jobs after gelus hc8-17, b0 (lands last) on the idle DVE.
   gamma2 + w2 quant emitted at the hc19 hook - strictly after all
   acc2 writers (the dep tracker links reads only to earlier writes).
 - fc1 hc-outer: 12 bf16 matmuls per (hc, super) into 2-bank PSUM
   tiles; one ACT Gelu (scale g1f or g1h per slice path, bias b1).
 - fc2 in dc-groups of (3,2,1); epilogues are single ACT jobs except
   the final group (split ACT/DVE for a ~1.2us tail); DMA out
   alternates gpsimd/sync queues.
 - x supertile 0 loaded after w1 in 6 kd-chunks; host upcasts y.
"""
import sys

for _p in ("/root/.axon_site", "/root/.axon_site/_ro/trn_rl_repo",
           "/root/.axon_site/_ro/pypackages", "/opt/trn_rl_repo"):
    if _p not in sys.path:
        sys.path.append(_p)

import os

import ml_dtypes
import numpy as np

# bisect flags (default = full v3.1 behavior)
_NO_ACT_CHUNK_ABS = os.environ.get("K_NO_ACT_CHUNK_ABS", "0") == "1"
_NO_SIGN_ASSIST = os.environ.get("K_NO_SIGN_ASSIST", "0") == "1"

from concourse import bacc
import concourse.mybir as mybir
from concourse import bass_isa
from concourse.tile import TileContext
from concourse.tile_rust import add_dep_helper
from concourse.bass_utils import run_bass_kernel_spmd

FP32 = mybir.dt.float32
FP16 = mybir.dt.float16
BF16 = mybir.dt.bfloat16
FP8 = mybir.dt.float8e4
Act = mybir.ActivationFunctionType
Alu = mybir.AluOpType
AxX = mybir.AxisListType.X
AxXY = mybir.AxisListType.XY

N_CORES = 8
B, S, D, H = 64, 197, 768, 3072
TOK = B * S                 # 12608
TOK_PER = TOK // N_CORES    # 1576
NS = 2                      # token supertiles per core
STN = TOK_PER // NS         # 788
TN = STN // 2               # 394 (psum bank region)
KD = D // 128               # 6
KH = H // 128               # 24
NSL = 6                     # w1 column-slice count (hc quads)
W1S = H // NSL              # 512
EPS = 1e-5


def build():
    nc = bacc.Bacc("TRN2", target_bir_lowering=False, debug=False)
    xt = nc.declare_dram_parameter("xt", [128, NS, KD, STN], BF16,
                                   isOutput=False)
    wt1 = nc.declare_dram_parameter("wt1", [NSL, 128, KD, W1S], FP16,
                                    isOutput=False)
    wt2 = nc.declare_dram_parameter("wt2", [H, D], FP32, isOutput=False)
    b1r = nc.declare_dram_parameter("b1r", [128, KH], FP32, isOutput=False)
    b2r = nc.declare_dram_parameter("b2r", [128, KD], FP32, isOutput=False)
    yt = nc.declare_dram_parameter("yt", [D, TOK_PER], BF16, isOutput=True)

    with TileContext(nc) as tc:
        with (
            tc.tile_pool(name="singles", bufs=1) as singles,
            tc.tile_pool(name="wbig", bufs=6) as wbig,   # w1 fp16 / w2 fp32
            tc.tile_pool(name="t1", bufs=12) as t1p,     # ternary w1 18K
            tc.tile_pool(name="t2", bufs=12) as t2p,     # ternary w2 18K
            tc.tile_pool(name="xp", bufs=1) as xp,       # x bf16 19K
            tc.tile_pool(name="hp", bufs=24) as hp,      # h bf16 38K
            tc.tile_pool(name="yp", bufs=3) as yp,       # y staging 4.7K
            tc.tile_pool(name="qp", bufs=2) as qp,       # quant transients
            tc.tile_pool(name="sp", bufs=3) as sgp,      # sign transients
            tc.tile_pool(name="dp", bufs=1) as dp,       # abs dumps
            tc.tile_pool(name="ps", bufs=4, space="PSUM") as psp,
        ):
            # warm the gpsimd custom-op library early
            dmy = singles.tile([128, 2], FP32, tag="dmy")
            nc.gpsimd.memset(dmy, 0.0)
            dmy2 = singles.tile([128, 1], FP32, tag="dmy2")
            nc.gpsimd.partition_all_reduce(dmy2, dmy[:, 0:1], channels=128,
                                           reduce_op=bass_isa.ReduceOp.add)
            # junk operands for the PE warm-up matmuls
            jl = singles.tile([128, 128], FP8, tag="jl")
            nc.gpsimd.memset(jl, 1.0)
            jr = singles.tile([128, TN], BF16, tag="jr")
            nc.gpsimd.memset(jr, 0.0)

            def gamma_chain(acc_cols, total_elems, tag):
                rsum = singles.tile([128, 1], FP32, tag=tag + "_rs")
                nc.vector.tensor_reduce(out=rsum[:, 0:1], in_=acc_cols,
                                        axis=AxX, op=Alu.add)
                allr = singles.tile([128, 1], FP32, tag=tag + "_ar")
                nc.gpsimd.partition_all_reduce(allr, rsum, channels=128,
                                               reduce_op=bass_isa.ReduceOp.add)
                # gn first: the quant b-pass / Sign biases need gn/gh
                gn = singles.tile([128, 1], FP32, tag=tag + "_gn")
                gn_i = nc.vector.tensor_scalar(
                    out=gn, in0=allr, scalar1=-0.5 / total_elems,
                    scalar2=-0.5 * EPS, op0=Alu.mult, op1=Alu.add)
                gh = singles.tile([128, 1], FP32, tag=tag + "_gh")
                nc.vector.tensor_scalar(
                    out=gh, in0=allr, scalar1=0.5 / total_elems,
                    scalar2=0.5 * EPS, op0=Alu.mult, op1=Alu.add)
                gf = singles.tile([128, 1], FP32, tag=tag + "_gf")
                nc.vector.tensor_scalar(
                    out=gf, in0=allr, scalar1=1.0 / total_elems,
                    scalar2=EPS, op0=Alu.mult, op1=Alu.add)
                return gf, gh, gn, gn_i

            def quant_dve(w_in, dst, gh, gn, btag, bdt, bbufs=1):
                """2-pass ternary -> {-1,0,1}: b=(w<=-g/2); t=(w>=g/2)-b"""
                b = qp.tile(list(w_in.shape), bdt, tag=btag, bufs=bbufs,
                            name="b")
                nc.vector.tensor_scalar(out=b, in0=w_in, scalar1=gn[:, 0:1],
                                        scalar2=0.0, op0=Alu.is_le,
                                        op1=Alu.add)
                return nc.vector.scalar_tensor_tensor(
                    out=dst, in0=w_in, scalar=gh[:, 0:1], in1=b,
                    op0=Alu.is_ge, op1=Alu.subtract)

            def quant_sign_act(w_in, gh, gn, stag):
                """ACT part of assisted quant: sa=sign(w-g/2), sb=sign(w+g/2)
                (biases: -gh == gn, -gn == gh)."""
                sa = sgp.tile(list(w_in.shape), FP8, tag=stag + "a",
                              name="sa", bufs=2)
                nc.scalar.activation(sa, w_in, Act.Sign, bias=gn[:, 0:1])
                sb = sgp.tile(list(w_in.shape), FP8, tag=stag + "b",
                              name="sb", bufs=2)
                nc.scalar.activation(sb, w_in, Act.Sign, bias=gh[:, 0:1])
                return sa, sb

            # ---- w1 fp16: chunked DMAs; reduces alternate DVE/ACT.
            # Slice 5 is split [3,2,1] kd so the last (gamma-critical)
            # chunk's reduce is short. ----
            # (slice, kd0, kd1, c0, c1) -- slice 5's tail split finer so
            # the last (gamma-critical) reduces are short and parallel
            chunks = []
            for s in range(NSL - 1):
                chunks += [(s, 0, 3, 0, W1S), (s, 3, 6, 0, W1S)]
            chunks += [(5, 0, 3, 0, W1S), (5, 3, 5, 0, W1S),
                       (5, 5, 6, 0, 256), (5, 5, 6, 256, W1S)]
            acc1 = singles.tile([128, len(chunks)], FP32, tag="acc1")
            rdump = dp.tile([128, 3, W1S], BF16, tag="rdump", bufs=1)
            w1t = [wbig.tile([128, KD, W1S], FP16, tag="wbig", name="wf")
                   for _ in range(NSL)]
            w1_dmas = []
            w1_reds = []
            for i, (s, k0, k1, c0, c1) in enumerate(chunks):
                ch = w1t[s][:, k0:k1, c0:c1]
                dma = nc.sync.dma_start(out=ch,
                                        in_=wt1[s, :, k0:k1, c0:c1])
                w1_dmas.append(dma)
                if i % 2 == 1 or _NO_ACT_CHUNK_ABS:
                    red = nc.vector.tensor_reduce(
                        out=acc1[:, i:i + 1], in_=ch, axis=AxXY,
                        op=Alu.add, apply_absolute_value=True)
                else:
                    red = nc.scalar.activation(
                        rdump[:, 0:k1 - k0, 0:c1 - c0], ch, Act.Abs,
                        accum_out=acc1[:, i:i + 1])
                w1_reds.append(red)

            b1sb = singles.tile([128, KH], FP32, tag="b1sb")
            d_b1 = nc.sync.dma_start(out=b1sb, in_=b1r[:, :])
            add_dep_helper(d_b1.ins, w1_dmas[0].ins, reason="b1 after w1c0")
            b2sb = singles.tile([128, KD], FP32, tag="b2sb")
            d_b2 = nc.sync.dma_start(out=b2sb, in_=b2r[:, :])
            add_dep_helper(d_b2.ins, w1_dmas[0].ins, reason="b2 after w1c0")

            # ---- junk matmuls: keep HAM warm while w1 streams ----
            jp = psp.tile([128, 2, 512], FP32, tag="ps")
            for i, red in enumerate(w1_reds):
                for j in range(3):
                    mm = nc.tensor.matmul(jp[:, 0, 0:TN], jl, jr,
                                          start=True, stop=True)
                    if j == 0:
                        add_dep_helper(mm.ins, red.ins,
                                       reason=f"warm batch {i}")

            g1f, g1h, g1n, g1n_i = gamma_chain(acc1, D * H, "g1")
            # bridge junk: covers gamma-chain -> first real matmul gap
            for j in range(2):
                mm = nc.tensor.matmul(jp[:, 0, 0:TN], jl, jr,
                                      start=True, stop=True)
                if j == 0:
                    add_dep_helper(mm.ins, g1n_i.ins, reason="warm bridge")

            # ---- x: supertile 0 in kd-chunks (after w1), then x1 ----
            xs = []
            x0 = xp.tile([128, KD, STN], BF16, tag="xs0")
            x_dmas = []
            for kd in range(KD):
                dma = nc.sync.dma_start(out=x0[:, kd, :],
                                        in_=xt[:, 0, kd, :])
                add_dep_helper(dma.ins, w1_dmas[-1].ins,
                               reason="x0 after w1 queued")
                x_dmas.append(dma)
            xs.append(x0)
            x1 = xp.tile([128, KD, STN], BF16, tag="xs1")
            d_x1 = nc.sync.dma_start(out=x1, in_=xt[:, 1, :, :])
            add_dep_helper(d_x1.ins, x_dmas[-1].ins, reason="x1 after x0")
            x_dmas.append(d_x1)
            xs.append(x1)

            # ---- w2 fp32 row-batches.  Issued b5..b0 so batch bt reuses
            # w1 slice (5-bt)'s slot: the slot-availability order (s0's
            # quant readers finish first) matches the DMA issue order,
            # and b0 -- consumed first by fc2 -- lands last but still
            # well before fc2(0). ----
            w2t = [None] * KD
            for bt in range(KD - 1, -1, -1):
                wf = wbig.tile([128, 4, D], FP32, tag="wbig", name="wf2")
                src = wt2[bt * 512:(bt + 1) * 512, :]
                dma = nc.sync.dma_start(
                    out=wf, in_=src.rearrange("(c p) f -> p c f", p=128))
                add_dep_helper(dma.ins, x_dmas[-1].ins,
                               reason="w2 after x")
                w2t[bt] = wf

            # acc2: cols 0-9 = ACT half-jobs (b5..b1); col 10 = DVE b0
            acc2 = singles.tile([128, 11], FP32, tag="acc2")

            # t1 per (slice, half): [128,3,512] fp8; t2 per (batch, half)
            t1sh = [[t1p.tile([128, 3, W1S], FP8, tag="t1", name="t1")
                     for _ in range(2)] for _ in range(NSL)]
            t2bh = [[t2p.tile([128, 4, 384], FP8, tag="t2", name="t2")
                     for _ in range(2)] for _ in range(KD)]

            def t1w(sl, kd, c0, c1):
                return t1sh[sl][kd // 3][:, kd % 3, c0:c1]

            # ---- slice0 col-quarter q0 (hc0): DVE2 kd-triple jobs;
            # q1 (hc1) is ACT-assisted in parallel so hc1's weights
            # land ~0.8us earlier than a serial DVE chain would ----
            for tr in range(2):
                quant_dve(w1t[0][:, 3 * tr:3 * tr + 3, 0:128],
                          t1sh[0][tr][:, :, 0:128],
                          g1h, g1n, "qb1q", FP16, bbufs=2)
            if not _NO_SIGN_ASSIST:
                q1_signs = [quant_sign_act(
                    w1t[0][:, 3 * tr:3 * tr + 3, 128:256],
                    g1h, g1n, "sq") for tr in range(2)]
            else:
                for tr in range(2):
                    quant_dve(w1t[0][:, 3 * tr:3 * tr + 3, 128:256],
                              t1sh[0][tr][:, :, 128:256],
                              g1h, g1n, "qb1q", FP16, bbufs=2)
            s1_signs = [None, None]  # per half-slice, set in hooks
            if _NO_SIGN_ASSIST:
                for tr in range(2):
                    quant_dve(w1t[0][:, 3 * tr:3 * tr + 3, 256:512],
                              t1sh[0][tr][:, :, 256:512], g1h, g1n,
                              "qb1q2", FP16)
                for hf in range(2):
                    quant_dve(w1t[1][:, 3 * hf:3 * hf + 3, :],
                              t1sh[1][hf], g1h, g1n, "qb1s1", FP16)
            else:
                # ---- slice0 cols 256:512 + slice1: ACT-assisted ----
                # ACT: sign-pairs per kd-triple (before gelu hc0)
                s0b_signs = []
                for tr in range(2):
                    s0b_signs.append(quant_sign_act(
                        w1t[0][:, 3 * tr:3 * tr + 3, 256:512],
                        g1h, g1n, "sh"))
                # slice1 half 0's signs also ahead of gelu hc0
                s1_signs[0] = quant_sign_act(
                    w1t[1][:, 0:3, :], g1h, g1n, "sf")
                # DVE combines: q1 first (hc1), then s0B (hc2/3)
                for tr in range(2):
                    sa, sb = q1_signs[tr]
                    nc.vector.tensor_tensor(
                        out=t1sh[0][tr][:, :, 128:256],
                        in0=sa, in1=sb, op=Alu.add)
                for tr in range(2):
                    sa, sb = s0b_signs[tr]
                    nc.vector.tensor_tensor(
                        out=t1sh[0][tr][:, :, 256:512],
                        in0=sa, in1=sb, op=Alu.add)

            # slice1 sign-pairs are interleaved with fc1(0) gelus; the
            # DVE combines + remaining quant jobs are emitted below via
            # deferred lists consumed by fc1(0)'s hc loop hooks.

            # ---- fc1 / fc2 ----
            hbt = {}

            def gelu_scale(hc):
                # slices quantized via ACT-assist hold {-2,0,2}
                if _NO_SIGN_ASSIST:
                    return g1f
                return g1h if 1 <= hc <= 7 else g1f

            def fc1(s, hooks=None):
                for hc in range(KH):
                    sl, off = hc // 4, (hc % 4) * 128
                    ps = psp.tile([128, 2, 512], FP32, tag="ps")
                    for kd in range(KD):
                        lhsT = t1w(sl, kd, off, off + 128)
                        for blk in range(2):
                            nc.tensor.matmul(
                                ps[:, blk, 0:TN], lhsT,
                                xs[s][:, kd, blk * TN:(blk + 1) * TN],
                                start=(kd == 0), stop=(kd == KD - 1))
                    hbv = hp.tile([128, 2, TN], BF16, tag="hb")
                    nc.scalar.activation(hbv, ps[:, :, 0:TN], Act.Gelu,
                                         bias=b1sb[:, hc:hc + 1],
                                         scale=gelu_scale(hc)[:, 0:1])
                    hbt.setdefault(s, []).append(hbv)
                    if hooks:
                        hooks(hc)

            def fc1_0_hooks(hc):
                # slice1 half 1's ACT sign-pair after gelu hc0
                if hc in (0, 1, 2):
                    if not _NO_SIGN_ASSIST and hc == 0:
                        s1_signs[1] = quant_sign_act(
                            w1t[1][:, 3:6, :], g1h, g1n, "sf")
                    if hc == 2:
                        # DVE combines for slice1 (chase the signs)
                        if not _NO_SIGN_ASSIST:
                            for hf in range(2):
                                sa, sb = s1_signs[hf]
                                nc.vector.tensor_tensor(
                                    out=t1sh[1][hf],
                                    in0=sa, in1=sb, op=Alu.add)
                        # slices 2-5: DVE2 half-slice jobs
                        last_q = None
                        for sl in range(2, NSL):
                            for hf in range(2):
                                last_q = quant_dve(
                                    w1t[sl][:, 3 * hf:3 * hf + 3, :],
                                    t1sh[sl][hf], g1h, g1n,
                                    "qb1", FP16)
                        # |w2| for b0 (lands last) on the then-idle DVE;
                        # forced after the w1 quant stream so its DMA
                        # wait can't head-of-line-block the quant
                        red_b0 = nc.vector.tensor_reduce(
                            out=acc2[:, 10:11], in_=w2t[0], axis=AxXY,
                            op=Alu.add, apply_absolute_value=True)
                        add_dep_helper(red_b0.ins, last_q.ins,
                                       reason="b0 abs after w1 quant")
                elif 8 <= hc <= 17:
                    # |w2| for b5..b1 on ACT, [128,2,768] half-jobs in
                    # landing order (same slot as rdump)
                    j = hc - 8
                    bt, half = 5 - j // 2, j % 2
                    rd2 = dp.tile([128, 2, D], BF16, tag="rdump", bufs=1,
                                  name="rd2")
                    nc.scalar.activation(
                        rd2, w2t[bt][:, 2 * half:2 * half + 2, :],
                        Act.Abs, accum_out=acc2[:, j:j + 1])
                if hc == 19:
                    # gamma2 + w2 quant: emitted only now, AFTER all
                    # acc2 writers exist (the dep tracker links a read
                    # only to writes emitted before it)
                    gam2 = gamma_chain(acc2, D * H, "g2")
                    g2_holder.append(gam2)
                    # both halves are consumed by fc2(0) (hf is the
                    # OUTPUT dc split); quant in landing order b5..b0,
                    # hf0 (needed first) before hf1
                    for hf2 in range(2):
                        for bt2 in range(KD - 1, -1, -1):
                            quant_dve(
                                w2t[bt2][:, :,
                                         hf2 * 384:(hf2 + 1) * 384],
                                t2bh[bt2][hf2], gam2[1], gam2[2],
                                "qb2", FP32)

            g2_holder = []

            # kh in w2-landing order (b5..b0): t2 quant for the first-
            # consumed batches is ready earliest
            kh_order = [kh for bt in range(KD - 1, -1, -1)
                        for kh in range(4 * bt, 4 * bt + 4)]

            def fc2(s, g2f_):
                for dc0, gsz in ((0, 3), (3, 2), (5, 1)):
                    pss = [psp.tile([128, 2, 512], FP32, tag="ps",
                                    name="ps") for _ in range(gsz)]
                    for ki, kh in enumerate(kh_order):
                        for i in range(gsz):
                            dc = dc0 + i
                            hf, col = (0, dc) if dc < 3 else (1, dc - 3)
                            lhsT = t2bh[kh // 4][hf][:, kh % 4,
                                                     col * 128:col * 128 + 128]
                            for blk in range(2):
                                nc.tensor.matmul(
                                    pss[i][:, blk, 0:TN], lhsT,
                                    hbt[s][kh][:, blk, :],
                                    start=(ki == 0), stop=(ki == KH - 1))
                    for i in range(gsz):
                        dc = dc0 + i
                        ysb = yp.tile([128, 2, TN], BF16, tag="ysb")
                        if s == 1 and gsz == 1:
                            # final group: split ACT/DVE for min tail
                            nc.scalar.activation(ysb[:, 0, :],
                                                 pss[i][:, 0, 0:TN],
                                                 Act.Identity,
                                                 bias=b2sb[:, dc:dc + 1],
                                                 scale=g2f_[:, 0:1])
                            nc.vector.tensor_scalar(
                                out=ysb[:, 1, :], in0=pss[i][:, 1, 0:TN],
                                scalar1=g2f_[:, 0:1],
                                scalar2=b2sb[:, dc:dc + 1],
                                op0=Alu.mult, op1=Alu.add)
                        else:
                            nc.scalar.activation(ysb, pss[i][:, :, 0:TN],
                                                 Act.Identity,
                                                 bias=b2sb[:, dc:dc + 1],
                                                 scale=g2f_[:, 0:1])
                        eng = nc.gpsimd if dc % 2 == 0 else nc.sync
                        eng.dma_start(
                            out=yt[dc * 128:(dc + 1) * 128,
                                   s * STN:(s + 1) * STN],
                            in_=ysb)
                del hbt[s]

            fc1(0, hooks=fc1_0_hooks)
            g2f = g2_holder[0][0]
            fc2(0, g2f)
            fc1(1)
            fc2(1, g2f)

    nc.compile()
    return nc


_NC = None


def _get_nc():
    global _NC
    if _NC is None:
        _NC = build()
    return _NC


def kernel(x, w1, b1, w2, b2, _trace=False, _trace_kwargs=None):
    nc = _get_nc()
    x = np.asarray(x, dtype=np.float32)
    w1 = np.asarray(w1, dtype=np.float32)
    b1 = np.asarray(b1, dtype=np.float32)
    w2 = np.asarray(w2, dtype=np.float32)
    b2 = np.asarray(b2, dtype=np.float32)
    x2 = np.ascontiguousarray(x.reshape(TOK, D).T).astype(ml_dtypes.bfloat16)
    # w1.T is [D, H]; wt1[s, p, k, c] = w1.T[k*128+p, s*512+c], fp16
    wt1 = np.ascontiguousarray(
        w1.T.astype(np.float16).reshape(KD, 128, NSL, W1S)
        .transpose(2, 1, 0, 3))
    wt2 = np.ascontiguousarray(w2.T)                    # [3072, 768]
    b1r = np.ascontiguousarray(b1.reshape(KH, 128).T)   # [128, 24]
    b2r = np.ascontiguousarray(b2.reshape(KD, 128).T)   # [128, 6]
    in_maps = []
    for c in range(N_CORES):
        xc = x2[:, c * TOK_PER:(c + 1) * TOK_PER]       # [768, 1576]
        xc = xc.reshape(KD, 128, NS, STN).transpose(1, 2, 0, 3)
        in_maps.append({
            "xt": np.ascontiguousarray(xc),
            "wt1": wt1, "wt2": wt2, "b1r": b1r, "b2r": b2r,
        })
    out = run_bass_kernel_spmd(nc, in_maps, list(range(N_CORES)),
                               trace=_trace, **(_trace_kwargs or {}))
    res = out.results
    yt = np.concatenate([res[c]["yt"].astype(np.float32) for c in
                         range(N_CORES)], axis=1)
    y = np.ascontiguousarray(yt.T).reshape(B, S, D)
    if _trace:
        return y, out
    return y


# revision 48
# speedup vs baseline: 1.0206x; 1.0206x over previous
"""Trainium2 Bass kernel for nn_Mlp_13099650253522 (BitNet-ternary dense MLP).

  h = gelu(x @ ter_quant(w1).T + b1);  y = h @ ter_quant(w2).T + b2
  ter_quant(w) = clip(round(w / g), -1, 1) * g,  g = mean(|w|) + 1e-5

v3.8 (8 NeuronCores, data-parallel over the 64*197=12608 tokens),
~230us vs 249.5us baseline:
 - w1 shipped as fp16 (host cast; ~0.007 rel err vs 2e-2 gate, verified
   offline) in a per-slice partition-contiguous layout; 14 chunk DMAs
   (slice5 tail split finest).  w1 critical DMA 9.4MB -> 4.7MB.
 - gamma1 |w| chunk-reduces alternate DVE / ACT(Abs+accum) so the
   reduction chases the DMA stream instead of serializing on DVE.
 - PE warm-up: junk matmuls gated on each chunk's reduce keep the HAM
   clock at 2.4GHz through the DMA phase (plus a bridge batch gated on
   the gamma chain).
 - Ternary quant, two paths:
     DVE2: b=(w<=-g/2); t=(w>=g/2)-b  -> {-1,0,1}, scale gamma.
     ACT-assist (slice0 cols 256:512 + slice1): ACT Sign(w-g/2) and
     Sign(w+g/2), DVE adds -> {-2,0,2}, gelu scale gamma/2.  This
     offloads DVE exactly where fc1's consumption outpaces it.
   Slice0 cols 0:256 in kd-triple column-quarter jobs (hc0/hc1 chase);
   slices 2-5 in [128,3,512] half-slice jobs.
 - w2 fp32, issued b5..b0 reusing w1 slots 0..5 (slot-availability
   order == issue order); fc2's kh loop consumes batches in LANDING
   order (b5 first) so t2 quant chases.  |w2| sums: b5..b1 on ACT
   half-jobs after gelus hc8-17, b0 (lands last) on the idle DVE.
   gamma2 + w2 quant emitted at the hc19 hook - strictly after all
   acc2 writers (the dep tracker links reads only to earlier writes).
 - fc1 hc-outer: 12 bf16 matmuls per (hc, super) into 2-bank PSUM
   tiles; one ACT Gelu (scale g1f or g1h per slice path, bias b1).
 - fc2 in dc-groups of (3,2,1); epilogues are single ACT jobs except
   the final group (split ACT/DVE for a ~1.2us tail); DMA out
   alternates gpsimd/sync queues.
 - x supertile 0 loaded after w1 in 6 kd-chunks; host upcasts y.
"""
import sys

for _p in ("/root/.axon_site", "/root/.axon_site/_ro/trn_rl_repo",
           "/root/.axon_site/_ro/pypackages", "/opt/trn_rl_repo"):
    if _p not in sys.path:
        sys.path.append(_p)

import os

import ml_dtypes
import numpy as np

# bisect flags (default = full v3.1 behavior)
_NO_ACT_CHUNK_ABS = os.environ.get("K_NO_ACT_CHUNK_ABS", "0") == "1"
_NO_SIGN_ASSIST = os.environ.get("K_NO_SIGN_ASSIST", "0") == "1"

from concourse import bacc
import concourse.mybir as mybir
from concourse import bass_isa
from concourse.tile import TileContext
from concourse.tile_rust import add_dep_helper
from concourse.bass_utils import run_bass_kernel_spmd

FP32 = mybir.dt.float32
FP16 = mybir.dt.float16
BF16 = mybir.dt.bfloat16
FP8 = mybir.dt.float8e4
Act = mybir.ActivationFunctionType
Alu = mybir.AluOpType
AxX = mybir.AxisListType.X
AxXY = mybir.AxisListType.XY

N_CORES = 8
B, S, D, H = 64, 197, 768, 3072
TOK = B * S                 # 12608
TOK_PER = TOK // N_CORES    # 1576
NS = 2                      # token supertiles per core
STN = TOK_PER // NS         # 788
TN = STN // 2               # 394 (psum bank region)
KD = D // 128               # 6
KH = H // 128               # 24
NSL = 6                     # w1 column-slice count (hc quads)
W1S = H // NSL              # 512
EPS = 1e-5


def build():
    nc = bacc.Bacc("TRN2", target_bir_lowering=False, debug=False)
    xt = nc.declare_dram_parameter("xt", [128, NS, KD, STN], BF16,
                                   isOutput=False)
    wt1 = nc.declare_dram_parameter("wt1", [NSL, 128, KD, W1S], FP16,
                                    isOutput=False)
    wt2 = nc.declare_dram_parameter("wt2", [H, D], FP32, isOutput=False)
    b1r = nc.declare_dram_parameter("b1r", [128, KH], FP32, isOutput=False)
    b2r = nc.declare_dram_parameter("b2r", [128, KD], FP32, isOutput=False)
    yt = nc.declare_dram_parameter("yt", [D, TOK_PER], BF16, isOutput=True)

    with TileContext(nc) as tc:
        with (
            tc.tile_pool(name="singles", bufs=1) as singles,
            tc.tile_pool(name="wbig", bufs=6) as wbig,   # w1 fp16 / w2 fp32
            tc.tile_pool(name="t1", bufs=12) as t1p,     # ternary w1 18K
            tc.tile_pool(name="t2", bufs=12) as t2p,     # ternary w2 18K
            tc.tile_pool(name="xp", bufs=1) as xp,       # x bf16 19K
            tc.tile_pool(name="hp", bufs=24) as hp,      # h bf16 38K
            tc.tile_pool(name="yp", bufs=3) as yp,       # y staging 4.7K
            tc.tile_pool(name="qp", bufs=2) as qp,       # quant transients
            tc.tile_pool(name="sp", bufs=3) as sgp,      # sign transients
            tc.tile_pool(name="dp", bufs=1) as dp,       # abs dumps
            tc.tile_pool(name="ps", bufs=4, space="PSUM") as psp,
        ):
            # warm the gpsimd custom-op library early
            dmy = singles.tile([128, 2], FP32, tag="dmy")
            nc.gpsimd.memset(dmy, 0.0)
            dmy2 = singles.tile([128, 1], FP32, tag="dmy2")
            nc.gpsimd.partition_all_reduce(dmy2, dmy[:, 0:1], channels=128,
                                           reduce_op=bass_isa.ReduceOp.add)
            # junk operands for the PE warm-up matmuls
            jl = singles.tile([128, 128], FP8, tag="jl")
            nc.gpsimd.memset(jl, 1.0)
            jr = singles.tile([128, TN], BF16, tag="jr")
            nc.gpsimd.memset(jr, 0.0)

            def gamma_chain(acc_cols, total_elems, tag):
                rsum = singles.tile([128, 1], FP32, tag=tag + "_rs")
                nc.vector.tensor_reduce(out=rsum[:, 0:1], in_=acc_cols,
                                        axis=AxX, op=Alu.add)
                allr = singles.tile([128, 1], FP32, tag=tag + "_ar")
                nc.gpsimd.partition_all_reduce(allr, rsum, channels=128,
                                               reduce_op=bass_isa.ReduceOp.add)
                # gn first: the quant b-pass / Sign biases need gn/gh
                gn = singles.tile([128, 1], FP32, tag=tag + "_gn")
                gn_i = nc.vector.tensor_scalar(
                    out=gn, in0=allr, scalar1=-0.5 / total_elems,
                    scalar2=-0.5 * EPS, op0=Alu.mult, op1=Alu.add)
                gh = singles.tile([128, 1], FP32, tag=tag + "_gh")
                nc.vector.tensor_scalar(
                    out=gh, in0=allr, scalar1=0.5 / total_elems,
                    scalar2=0.5 * EPS, op0=Alu.mult, op1=Alu.add)
                gf = singles.tile([128, 1], FP32, tag=tag + "_gf")
                nc.vector.tensor_scalar(
                    out=gf, in0=allr, scalar1=1.0 / total_elems,
                    scalar2=EPS, op0=Alu.mult, op1=Alu.add)
                return gf, gh, gn, gn_i

            def quant_dve(w_in, dst, gh, gn, btag, bdt, bbufs=1):
                """2-pass ternary -> {-1,0,1}: b=(w<=-g/2); t=(w>=g/2)-b"""
                b = qp.tile(list(w_in.shape), bdt, tag=btag, bufs=bbufs,
                            name="b")
                nc.vector.tensor_scalar(out=b, in0=w_in, scalar1=gn[:, 0:1],
                                        scalar2=0.0, op0=Alu.is_le,
                                        op1=Alu.add)
                return nc.vector.scalar_tensor_tensor(
                    out=dst, in0=w_in, scalar=gh[:, 0:1], in1=b,
                    op0=Alu.is_ge, op1=Alu.subtract)

            def quant_sign_act(w_in, gh, gn, stag):
                """ACT part of assisted quant: sa=sign(w-g/2), sb=sign(w+g/2)
                (biases: -gh == gn, -gn == gh)."""
                sa = sgp.tile(list(w_in.shape), FP8, tag=stag + "a",
                              name="sa", bufs=2)
                nc.scalar.activation(sa, w_in, Act.Sign, bias=gn[:, 0:1])
                sb = sgp.tile(list(w_in.shape), FP8, tag=stag + "b",
                              name="sb", bufs=2)
                nc.scalar.activation(sb, w_in, Act.Sign, bias=gh[:, 0:1])
                return sa, sb

            # ---- w1 fp16: chunked DMAs; reduces alternate DVE/ACT.
            # Slice 5 is split [3,2,1] kd so the last (gamma-critical)
            # chunk's reduce is short. ----
            # (slice, kd0, kd1, c0, c1) -- slice 5's tail split finer so
            # the last (gamma-critical) reduces are short and parallel
            chunks = []
            for s in range(NSL - 1):
                chunks += [(s, 0, 3, 0, W1S), (s, 3, 6, 0, W1S)]
            chunks += [(5, 0, 3, 0, W1S), (5, 3, 5, 0, W1S),
                       (5, 5, 6, 0, 256), (5, 5, 6, 256, W1S)]
            acc1 = singles.tile([128, len(chunks)], FP32, tag="acc1")
            rdump = dp.tile([128, 3, W1S], BF16, tag="rdump", bufs=1)
            w1t = [wbig.tile([128, KD, W1S], FP16, tag="wbig", name="wf")
                   for _ in range(NSL)]
            w1_dmas = []
            w1_reds = []
            for i, (s, k0, k1, c0, c1) in enumerate(chunks):
                ch = w1t[s][:, k0:k1, c0:c1]
                dma = nc.sync.dma_start(out=ch,
                                        in_=wt1[s, :, k0:k1, c0:c1])
                w1_dmas.append(dma)
                if i % 2 == 1 or _NO_ACT_CHUNK_ABS:
                    red = nc.vector.tensor_reduce(
                        out=acc1[:, i:i + 1], in_=ch, axis=AxXY,
                        op=Alu.add, apply_absolute_value=True)
                else:
                    red = nc.scalar.activation(
                        rdump[:, 0:k1 - k0, 0:c1 - c0], ch, Act.Abs,
                        accum_out=acc1[:, i:i + 1])
                w1_reds.append(red)

            b1sb = singles.tile([128, KH], FP32, tag="b1sb")
            d_b1 = nc.sync.dma_start(out=b1sb, in_=b1r[:, :])
            add_dep_helper(d_b1.ins, w1_dmas[0].ins, reason="b1 after w1c0")
            b2sb = singles.tile([128, KD], FP32, tag="b2sb")
            d_b2 = nc.sync.dma_start(out=b2sb, in_=b2r[:, :])
            add_dep_helper(d_b2.ins, w1_dmas[0].ins, reason="b2 after w1c0")

            # ---- junk matmuls: keep HAM warm while w1 streams ----
            jp = psp.tile([128, 2, 512], FP32, tag="ps")
            for i, red in enumerate(w1_reds):
                for j in range(3):
                    mm = nc.tensor.matmul(jp[:, 0, 0:TN], jl, jr,
                                          start=True, stop=True)
                    if j == 0:
                        add_dep_helper(mm.ins, red.ins,
                                       reason=f"warm batch {i}")

            g1f, g1h, g1n, g1n_i = gamma_chain(acc1, D * H, "g1")
            # bridge junk: covers gamma-chain -> first real matmul gap
            for j in range(2):
                mm = nc.tensor.matmul(jp[:, 0, 0:TN], jl, jr,
                                      start=True, stop=True)
                if j == 0:
                    add_dep_helper(mm.ins, g1n_i.ins, reason="warm bridge")

            # ---- x: supertile 0 in kd-chunks (after w1), then x1 ----
            xs = []
            x0 = xp.tile([128, KD, STN], BF16, tag="xs0")
            x_dmas = []
            for kd in range(KD):
                dma = nc.sync.dma_start(out=x0[:, kd, :],
                                        in_=xt[:, 0, kd, :])
                add_dep_helper(dma.ins, w1_dmas[-1].ins,
                               reason="x0 after w1 queued")
                x_dmas.append(dma)
            xs.append(x0)
            x1 = xp.tile([128, KD, STN], BF16, tag="xs1")
            d_x1 = nc.sync.dma_start(out=x1, in_=xt[:, 1, :, :])
            add_dep_helper(d_x1.ins, x_dmas[-1].ins, reason="x1 after x0")
            x_dmas.append(d_x1)
            xs.append(x1)

            # ---- w2 fp32 row-batches.  Issued b5..b0 so batch bt reuses
            # w1 slice (5-bt)'s slot: the slot-availability order (s0's
            # quant readers finish first) matches the DMA issue order,
            # and b0 -- consumed first by fc2 -- lands last but still
            # well before fc2(0). ----
            w2t = [None] * KD
            for bt in range(KD - 1, -1, -1):
                wf = wbig.tile([128, 4, D], FP32, tag="wbig", name="wf2")
                src = wt2[bt * 512:(bt + 1) * 512, :]
                dma = nc.sync.dma_start(
                    out=wf, in_=src.rearrange("(c p) f -> p c f", p=128))
                add_dep_helper(dma.ins, x_dmas[-1].ins,
                               reason="w2 after x")
                w2t[bt] = wf

            # acc2: cols 0-9 = ACT half-jobs (b5..b1); col 10 = DVE b0
            acc2 = singles.tile([128, 11], FP32, tag="acc2")

            # t1 per (slice, half): [128,3,512] fp8; t2 per (batch, half)
            t1sh = [[t1p.tile([128, 3, W1S], FP8, tag="t1", name="t1")
                     for _ in range(2)] for _ in range(NSL)]
            t2bh = [[t2p.tile([128, 4, 384], FP8, tag="t2", name="t2")
                     for _ in range(2)] for _ in range(KD)]

            def t1w(sl, kd, c0, c1):
                return t1sh[sl][kd // 3][:, kd % 3, c0:c1]

            for q in range(2):
                for tr in range(2):
                    quant_dve(
                        w1t[0][:, 3 * tr:3 * tr + 3,
                               q * 128:q * 128 + 128],
                        t1sh[0][tr][:, :, q * 128:q * 128 + 128],
                        g1h, g1n, "qb1q", FP16, bbufs=2)
            s1_signs = [None, None]  # per half-slice, set in hooks
            if _NO_SIGN_ASSIST:
                for tr in range(2):
                    quant_dve(w1t[0][:, 3 * tr:3 * tr + 3, 256:512],
                              t1sh[0][tr][:, :, 256:512], g1h, g1n,
                              "qb1q2", FP16)
                for hf in range(2):
                    quant_dve(w1t[1][:, 3 * hf:3 * hf + 3, :],
                              t1sh[1][hf], g1h, g1n, "qb1s1", FP16)
            else:
                # ---- slice0 cols 256:512 + slice1: ACT-assisted ----
                # ACT: sign-pairs per kd-triple (before gelu hc0)
                s0b_signs = []
                for tr in range(2):
                    s0b_signs.append(quant_sign_act(
                        w1t[0][:, 3 * tr:3 * tr + 3, 256:512],
                        g1h, g1n, "sh"))
                # slice1 half 0's signs also ahead of gelu hc0
                s1_signs[0] = quant_sign_act(
                    w1t[1][:, 0:3, :], g1h, g1n, "sf")
                # DVE combine for s0B
                for tr in range(2):
                    sa, sb = s0b_signs[tr]
                    nc.vector.tensor_tensor(
                        out=t1sh[0][tr][:, :, 256:512],
                        in0=sa, in1=sb, op=Alu.add)

            # slice1 sign-pairs are interleaved with fc1(0) gelus; the
            # DVE combines + remaining quant jobs are emitted below via
            # deferred lists consumed by fc1(0)'s hc loop hooks.

            # ---- fc1 / fc2 ----
            hbt = {}

            def gelu_scale(hc):
                # slices quantized via ACT-assist hold {-2,0,2}
                if _NO_SIGN_ASSIST:
                    return g1f
                return g1h if 2 <= hc <= 7 else g1f

            def fc1(s, hooks=None):
                for hc in range(KH):
                    sl, off = hc // 4, (hc % 4) * 128
                    ps = psp.tile([128, 2, 512], FP32, tag="ps")
                    for kd in range(KD):
                        lhsT = t1w(sl, kd, off, off + 128)
                        for blk in range(2):
                            nc.tensor.matmul(
                                ps[:, blk, 0:TN], lhsT,
                                xs[s][:, kd, blk * TN:(blk + 1) * TN],
                                start=(kd == 0), stop=(kd == KD - 1))
                    hbv = hp.tile([128, 2, TN], BF16, tag="hb")
                    nc.scalar.activation(hbv, ps[:, :, 0:TN], Act.Gelu,
                                         bias=b1sb[:, hc:hc + 1],
                                         scale=gelu_scale(hc)[:, 0:1])
                    hbt.setdefault(s, []).append(hbv)
                    if hooks:
                        hooks(hc)

            def fc1_0_hooks(hc):
                # slice1 half 1's ACT sign-pair after gelu hc0
                if hc in (0, 1, 2):
                    if not _NO_SIGN_ASSIST and hc == 0:
                        s1_signs[1] = quant_sign_act(
                            w1t[1][:, 3:6, :], g1h, g1n, "sf")
                    if hc == 2:
                        # DVE combines for slice1 (chase the signs)
                        if not _NO_SIGN_ASSIST:
                            for hf in range(2):
                                sa, sb = s1_signs[hf]
                                nc.vector.tensor_tensor(
                                    out=t1sh[1][hf],
                                    in0=sa, in1=sb, op=Alu.add)
                        # slices 2-5: DVE2 half-slice jobs
                        last_q = None
                        for sl in range(2, NSL):
                            for hf in range(2):
                                last_q = quant_dve(
                                    w1t[sl][:, 3 * hf:3 * hf + 3, :],
                                    t1sh[sl][hf], g1h, g1n,
                                    "qb1", FP16)
                        # |w2| for b0 (lands last) on the then-idle DVE;
                        # forced after the w1 quant stream so its DMA
                        # wait can't head-of-line-block the quant
                        red_b0 = nc.vector.tensor_reduce(
                            out=acc2[:, 10:11], in_=w2t[0], axis=AxXY,
                            op=Alu.add, apply_absolute_value=True)
                        add_dep_helper(red_b0.ins, last_q.ins,
                                       reason="b0 abs after w1 quant")
                elif 8 <= hc <= 17:
                    # |w2| for b5..b1 on ACT, [128,2,768] half-jobs in
                    # landing order (same slot as rdump)
                    j = hc - 8
                    bt, half = 5 - j // 2, j % 2
                    rd2 = dp.tile([128, 2, D], BF16, tag="rdump", bufs=1,
                                  name="rd2")
                    nc.scalar.activation(
                        rd2, w2t[bt][:, 2 * half:2 * half + 2, :],
                        Act.Abs, accum_out=acc2[:, j:j + 1])
                if hc == 19:
                    # gamma2 + w2 quant: emitted only now, AFTER all
                    # acc2 writers exist (the dep tracker links a read
                    # only to writes emitted before it)
                    gam2 = gamma_chain(acc2, D * H, "g2")
                    g2_holder.append(gam2)
                    # both halves are consumed by fc2(0) (hf is the
                    # OUTPUT dc split); quant in landing order b5..b0,
                    # hf0 (needed first) before hf1
                    for hf2 in range(2):
                        for bt2 in range(KD - 1, -1, -1):
                            quant_dve(
                                w2t[bt2][:, :,
                                         hf2 * 384:(hf2 + 1) * 384],
                                t2bh[bt2][hf2], gam2[1], gam2[2],
                                "qb2", FP32)

            g2_holder = []

            # kh in w2-landing order (b5..b0): t2 quant for the first-
            # consumed batches is ready earliest
            kh_order = [kh for bt in range(KD - 1, -1, -1)
                        for kh in range(4 * bt, 4 * bt + 4)]

            def fc2(s, g2f_):
                for dc0, gsz in ((0, 3), (3, 2), (5, 1)):
                    pss = [psp.tile([128, 2, 512], FP32, tag="ps",
                                    name="ps") for _ in range(gsz)]
                    for ki, kh in enumerate(kh_order):
                        for i in range(gsz):
                            dc = dc0 + i
                            hf, col = (0, dc) if dc < 3 else (1, dc - 3)
                            lhsT = t2bh[kh // 4][hf][:, kh % 4,
                                                     col * 128:col * 128 + 128]
                            for blk in range(2):
                                nc.tensor.matmul(
                                    pss[i][:, blk, 0:TN], lhsT,
                                    hbt[s][kh][:, blk, :],
                                    start=(ki == 0), stop=(ki == KH - 1))
                    for i in range(gsz):
                        dc = dc0 + i
                        ysb = yp.tile([128, 2, TN], BF16, tag="ysb")
                        if s == 1 and gsz == 1:
                            # final group: split ACT/DVE for min tail
                            nc.scalar.activation(ysb[:, 0, :],
                                                 pss[i][:, 0, 0:TN],
                                                 Act.Identity,
                                                 bias=b2sb[:, dc:dc + 1],
                                                 scale=g2f_[:, 0:1])
                            nc.vector.tensor_scalar(
                                out=ysb[:, 1, :], in0=pss[i][:, 1, 0:TN],
                                scalar1=g2f_[:, 0:1],
                                scalar2=b2sb[:, dc:dc + 1],
                                op0=Alu.mult, op1=Alu.add)
                        else:
                            nc.scalar.activation(ysb, pss[i][:, :, 0:TN],
                                                 Act.Identity,
                                                 bias=b2sb[:, dc:dc + 1],
                                                 scale=g2f_[:, 0:1])
                        eng = nc.gpsimd if dc % 2 == 0 else nc.sync
                        eng.dma_start(
                            out=yt[dc * 128:(dc + 1) * 128,
                                   s * STN:(s + 1) * STN],
                            in_=ysb)
                del hbt[s]

            fc1(0, hooks=fc1_0_hooks)
            g2f = g2_holder[0][0]
            fc2(0, g2f)
            fc1(1)
            fc2(1, g2f)

    nc.compile()
    return nc


_NC = None


def _get_nc():
    global _NC
    if _NC is None:
        _NC = build()
    return _NC


def kernel(x, w1, b1, w2, b2, _trace=False, _trace_kwargs=None):
    nc = _get_nc()
    x = np.asarray(x, dtype=np.float32)
    w1 = np.asarray(w1, dtype=np.float32)
    b1 = np.asarray(b1, dtype=np.float32)
    w2 = np.asarray(w2, dtype=np.float32)
    b2 = np.asarray(b2, dtype=np.float32)
    x2 = np.ascontiguousarray(x.reshape(TOK, D).T).astype(ml_dtypes.bfloat16)
    # w1.T is [D, H]; wt1[s, p, k, c] = w1.T[k*128+p, s*512+c], fp16
    wt1 = np.ascontiguousarray(
        w1.T.astype(np.float16).reshape(KD, 128, NSL, W1S)
        .transpose(2, 1, 0, 3))
    wt2 = np.ascontiguousarray(w2.T)                    # [3072, 768]
    b1r = np.ascontiguousarray(b1.reshape(KH, 128).T)   # [128, 24]
    b2r = np.ascontiguousarray(b2.reshape(KD, 128).T)   # [128, 6]
    in_maps = []
    for c in range(N_CORES):
        xc = x2[:, c * TOK_PER:(c + 1) * TOK_PER]       # [768, 1576]
        xc = xc.reshape(KD, 128, NS, STN).transpose(1, 2, 0, 3)
        in_maps.append({
            "xt": np.ascontiguousarray(xc),
            "wt1": wt1, "wt2": wt2, "b1r": b1r, "b2r": b2r,
        })
    out = run_bass_kernel_spmd(nc, in_maps, list(range(N_CORES)),
                               trace=_trace, **(_trace_kwargs or {}))
    res = out.results
    yt = np.concatenate([res[c]["yt"].astype(np.float32) for c in
                         range(N_CORES)], axis=1)
    y = np.ascontiguousarray(yt.T).reshape(B, S, D)
    if _trace:
        return y, out
    return y


# revision 49
# speedup vs baseline: 1.0238x; 1.0031x over previous
"""Trainium2 Bass kernel for nn_Mlp_13099650253522 (BitNet-ternary dense MLP).

  h = gelu(x @ ter_quant(w1).T + b1);  y = h @ ter_quant(w2).T + b2
  ter_quant(w) = clip(round(w / g), -1, 1) * g,  g = mean(|w|) + 1e-5

v3.8 (8 NeuronCores, data-parallel over the 64*197=12608 tokens),
~230us vs 249.5us baseline:
 - w1 shipped as fp16 (host cast; ~0.007 rel err vs 2e-2 gate, verified
   offline) in a per-slice partition-contiguous layout; 14 chunk DMAs
   (slice5 tail split finest).  w1 critical DMA 9.4MB -> 4.7MB.
 - gamma1 |w| chunk-reduces alternate DVE / ACT(Abs+accum) so the
   reduction chases the DMA stream instead of serializing on DVE.
 - PE warm-up: junk matmuls gated on each chunk's reduce keep the HAM
   clock at 2.4GHz through the DMA phase (plus a bridge batch gated on
   the gamma chain).
 - Ternary quant, two paths:
     DVE2: b=(w<=-g/2); t=(w>=g/2)-b  -> {-1,0,1}, scale gamma.
     ACT-assist (slice0 cols 256:512 + slice1): ACT Sign(w-g/2) and
     Sign(w+g/2), DVE adds -> {-2,0,2}, gelu scale gamma/2.  This
     offloads DVE exactly where fc1's consumption outpaces it.
   Slice0 cols 0:256 in kd-triple column-quarter jobs (hc0/hc1 chase);
   slices 2-5 in [128,3,512] half-slice jobs.
 - w2 fp32, issued b5..b0 reusing w1 slots 0..5 (slot-availability
   order == issue order); fc2's kh loop consumes batches in LANDING
   order (b5 first) so t2 quant chases.  |w2| sums: b5..b1 on ACT
   half-jobs after gelus hc8-17, b0 (lands last) on the idle DVE.
   gamma2 + w2 quant emitted at the hc19 hook - strictly after all
   acc2 writers (the dep tracker links reads only to earlier writes).
 - fc1 hc-outer: 12 bf16 matmuls per (hc, super) into 2-bank PSUM
   tiles; one ACT Gelu (scale g1f or g1h per slice path, bias b1).
 - fc2 in dc-groups of (3,2,1); epilogues are single ACT jobs except
   the final group (split ACT/DVE for a ~1.2us tail); DMA out
   alternates gpsimd/sync queues.
 - x supertile 0 loaded after w1 in 6 kd-chunks; host upcasts y.
"""
import sys

for _p in ("/root/.axon_site", "/root/.axon_site/_ro/trn_rl_repo",
           "/root/.axon_site/_ro/pypackages", "/opt/trn_rl_repo"):
    if _p not in sys.path:
        sys.path.append(_p)

import os

import ml_dtypes
import numpy as np

# bisect flags (default = full v3.1 behavior)
_NO_ACT_CHUNK_ABS = os.environ.get("K_NO_ACT_CHUNK_ABS", "0") == "1"
_NO_SIGN_ASSIST = os.environ.get("K_NO_SIGN_ASSIST", "0") == "1"

from concourse import bacc
import concourse.mybir as mybir
from concourse import bass_isa
from concourse.tile import TileContext
from concourse.tile_rust import add_dep_helper
from concourse.bass_utils import run_bass_kernel_spmd

FP32 = mybir.dt.float32
FP16 = mybir.dt.float16
BF16 = mybir.dt.bfloat16
FP8 = mybir.dt.float8e4
Act = mybir.ActivationFunctionType
Alu = mybir.AluOpType
AxX = mybir.AxisListType.X
AxXY = mybir.AxisListType.XY

N_CORES = 8
B, S, D, H = 64, 197, 768, 3072
TOK = B * S                 # 12608
TOK_PER = TOK // N_CORES    # 1576
NS = 2                      # token supertiles per core
STN = TOK_PER // NS         # 788
TN = STN // 2               # 394 (psum bank region)
KD = D // 128               # 6
KH = H // 128               # 24
NSL = 6                     # w1 column-slice count (hc quads)
W1S = H // NSL              # 512
EPS = 1e-5


def build():
    nc = bacc.Bacc("TRN2", target_bir_lowering=False, debug=False)
    xt = nc.declare_dram_parameter("xt", [128, NS, KD, STN], BF16,
                                   isOutput=False)
    wt1 = nc.declare_dram_parameter("wt1", [NSL, 128, KD, W1S], FP16,
                                    isOutput=False)
    wt2 = nc.declare_dram_parameter("wt2", [H, D], FP32, isOutput=False)
    b1r = nc.declare_dram_parameter("b1r", [128, KH], FP32, isOutput=False)
    b2r = nc.declare_dram_parameter("b2r", [128, KD], FP32, isOutput=False)
    yt = nc.declare_dram_parameter("yt", [D, TOK_PER], BF16, isOutput=True)

    with TileContext(nc) as tc:
        with (
            tc.tile_pool(name="singles", bufs=1) as singles,
            tc.tile_pool(name="wbig", bufs=6) as wbig,   # w1 fp16 / w2 fp32
            tc.tile_pool(name="t1", bufs=12) as t1p,     # ternary w1 18K
            tc.tile_pool(name="t2", bufs=12) as t2p,     # ternary w2 18K
            tc.tile_pool(name="xp", bufs=1) as xp,       # x bf16 19K
            tc.tile_pool(name="hp", bufs=24) as hp,      # h bf16 38K
            tc.tile_pool(name="yp", bufs=3) as yp,       # y staging 4.7K
            tc.tile_pool(name="qp", bufs=2) as qp,       # quant transients
            tc.tile_pool(name="sp", bufs=3) as sgp,      # sign transients
            tc.tile_pool(name="dp", bufs=1) as dp,       # abs dumps
            tc.tile_pool(name="ps", bufs=4, space="PSUM") as psp,
        ):
            # warm the gpsimd custom-op library early
            dmy = singles.tile([128, 2], FP32, tag="dmy")
            nc.gpsimd.memset(dmy, 0.0)
            dmy2 = singles.tile([128, 1], FP32, tag="dmy2")
            nc.gpsimd.partition_all_reduce(dmy2, dmy[:, 0:1], channels=128,
                                           reduce_op=bass_isa.ReduceOp.add)
            # junk operands for the PE warm-up matmuls
            jl = singles.tile([128, 128], FP8, tag="jl")
            nc.gpsimd.memset(jl, 1.0)
            jr = singles.tile([128, TN], BF16, tag="jr")
            nc.gpsimd.memset(jr, 0.0)

            def gamma_chain(acc_cols, total_elems, tag):
                rsum = singles.tile([128, 1], FP32, tag=tag + "_rs")
                nc.vector.tensor_reduce(out=rsum[:, 0:1], in_=acc_cols,
                                        axis=AxX, op=Alu.add)
                allr = singles.tile([128, 1], FP32, tag=tag + "_ar")
                nc.gpsimd.partition_all_reduce(allr, rsum, channels=128,
                                               reduce_op=bass_isa.ReduceOp.add)
                # gn first: the quant b-pass / Sign biases need gn/gh
                gn = singles.tile([128, 1], FP32, tag=tag + "_gn")
                gn_i = nc.vector.tensor_scalar(
                    out=gn, in0=allr, scalar1=-0.5 / total_elems,
                    scalar2=-0.5 * EPS, op0=Alu.mult, op1=Alu.add)
                gh = singles.tile([128, 1], FP32, tag=tag + "_gh")
                nc.vector.tensor_scalar(
                    out=gh, in0=allr, scalar1=0.5 / total_elems,
                    scalar2=0.5 * EPS, op0=Alu.mult, op1=Alu.add)
                gf = singles.tile([128, 1], FP32, tag=tag + "_gf")
                nc.vector.tensor_scalar(
                    out=gf, in0=allr, scalar1=1.0 / total_elems,
                    scalar2=EPS, op0=Alu.mult, op1=Alu.add)
                return gf, gh, gn, gn_i

            def quant_dve(w_in, dst, gh, gn, btag, bdt, bbufs=1):
                """2-pass ternary -> {-1,0,1}: b=(w<=-g/2); t=(w>=g/2)-b"""
                b = qp.tile(list(w_in.shape), bdt, tag=btag, bufs=bbufs,
                            name="b")
                nc.vector.tensor_scalar(out=b, in0=w_in, scalar1=gn[:, 0:1],
                                        scalar2=0.0, op0=Alu.is_le,
                                        op1=Alu.add)
                return nc.vector.scalar_tensor_tensor(
                    out=dst, in0=w_in, scalar=gh[:, 0:1], in1=b,
                    op0=Alu.is_ge, op1=Alu.subtract)

            def quant_sign_act(w_in, gh, gn, stag):
                """ACT part of assisted quant: sa=sign(w-g/2), sb=sign(w+g/2)
                (biases: -gh == gn, -gn == gh)."""
                sa = sgp.tile(list(w_in.shape), FP8, tag=stag + "a",
                              name="sa", bufs=2)
                nc.scalar.activation(sa, w_in, Act.Sign, bias=gn[:, 0:1])
                sb = sgp.tile(list(w_in.shape), FP8, tag=stag + "b",
                              name="sb", bufs=2)
                nc.scalar.activation(sb, w_in, Act.Sign, bias=gh[:, 0:1])
                return sa, sb

            # ---- w1 fp16: chunked DMAs; reduces alternate DVE/ACT.
            # Slice 5 is split [3,2,1] kd so the last (gamma-critical)
            # chunk's reduce is short. ----
            # (slice, kd0, kd1, c0, c1) -- slice 5's tail split finer so
            # the last (gamma-critical) reduces are short and parallel
            chunks = []
            for s in range(NSL - 1):
                chunks += [(s, 0, 3, 0, W1S), (s, 3, 6, 0, W1S)]
            chunks += [(5, 0, 3, 0, W1S), (5, 3, 5, 0, W1S),
                       (5, 5, 6, 0, 256), (5, 5, 6, 256, W1S)]
            acc1 = singles.tile([128, len(chunks)], FP32, tag="acc1")
            rdump = dp.tile([128, 3, W1S], BF16, tag="rdump", bufs=1)
            w1t = [wbig.tile([128, KD, W1S], FP16, tag="wbig", name="wf")
                   for _ in range(NSL)]
            w1_dmas = []
            w1_reds = []
            for i, (s, k0, k1, c0, c1) in enumerate(chunks):
                ch = w1t[s][:, k0:k1, c0:c1]
                dma = nc.sync.dma_start(out=ch,
                                        in_=wt1[s, :, k0:k1, c0:c1])
                w1_dmas.append(dma)
                if i % 2 == 1 or _NO_ACT_CHUNK_ABS:
                    red = nc.vector.tensor_reduce(
                        out=acc1[:, i:i + 1], in_=ch, axis=AxXY,
                        op=Alu.add, apply_absolute_value=True)
                else:
                    red = nc.scalar.activation(
                        rdump[:, 0:k1 - k0, 0:c1 - c0], ch, Act.Abs,
                        accum_out=acc1[:, i:i + 1])
                w1_reds.append(red)

            b1sb = singles.tile([128, KH], FP32, tag="b1sb")
            d_b1 = nc.sync.dma_start(out=b1sb, in_=b1r[:, :])
            add_dep_helper(d_b1.ins, w1_dmas[0].ins, reason="b1 after w1c0")
            b2sb = singles.tile([128, KD], FP32, tag="b2sb")
            d_b2 = nc.sync.dma_start(out=b2sb, in_=b2r[:, :])
            add_dep_helper(d_b2.ins, w1_dmas[0].ins, reason="b2 after w1c0")

            # ---- junk matmuls: keep HAM warm while w1 streams ----
            jp = psp.tile([128, 2, 512], FP32, tag="ps")
            for i, red in enumerate(w1_reds):
                for j in range(3):
                    mm = nc.tensor.matmul(jp[:, 0, 0:TN], jl, jr,
                                          start=True, stop=True)
                    if j == 0:
                        add_dep_helper(mm.ins, red.ins,
                                       reason=f"warm batch {i}")

            g1f, g1h, g1n, g1n_i = gamma_chain(acc1, D * H, "g1")
            # bridge junk: covers gamma-chain -> first real matmul gap
            for j in range(2):
                mm = nc.tensor.matmul(jp[:, 0, 0:TN], jl, jr,
                                      start=True, stop=True)
                if j == 0:
                    add_dep_helper(mm.ins, g1n_i.ins, reason="warm bridge")

            # ---- x: supertile 0 in kd-chunks (after w1), then x1 ----
            xs = []
            x0 = xp.tile([128, KD, STN], BF16, tag="xs0")
            x_dmas = []
            for kd in range(KD):
                dma = nc.sync.dma_start(out=x0[:, kd, :],
                                        in_=xt[:, 0, kd, :])
                add_dep_helper(dma.ins, w1_dmas[-1].ins,
                               reason="x0 after w1 queued")
                x_dmas.append(dma)
            xs.append(x0)
            x1 = xp.tile([128, KD, STN], BF16, tag="xs1")
            d_x1 = nc.sync.dma_start(out=x1, in_=xt[:, 1, :, :])
            add_dep_helper(d_x1.ins, x_dmas[-1].ins, reason="x1 after x0")
            x_dmas.append(d_x1)
            xs.append(x1)

            # ---- w2 fp32 row-batches.  Issued b5..b0 so batch bt reuses
            # w1 slice (5-bt)'s slot: the slot-availability order (s0's
            # quant readers finish first) matches the DMA issue order,
            # and b0 -- consumed first by fc2 -- lands last but still
            # well before fc2(0). ----
            w2t = [None] * KD
            for bt in range(KD - 1, -1, -1):
                wf = wbig.tile([128, 4, D], FP32, tag="wbig", name="wf2")
                src = wt2[bt * 512:(bt + 1) * 512, :]
                dma = nc.sync.dma_start(
                    out=wf, in_=src.rearrange("(c p) f -> p c f", p=128))
                add_dep_helper(dma.ins, x_dmas[-1].ins,
                               reason="w2 after x")
                w2t[bt] = wf

            # acc2: cols 0-9 = ACT half-jobs (b5..b1); col 10 = DVE b0
            acc2 = singles.tile([128, 11], FP32, tag="acc2")

            # t1 per (slice, half): [128,3,512] fp8; t2 per (batch, half)
            t1sh = [[t1p.tile([128, 3, W1S], FP8, tag="t1", name="t1")
                     for _ in range(2)] for _ in range(NSL)]
            t2bh = [[t2p.tile([128, 4, 384], FP8, tag="t2", name="t2")
                     for _ in range(2)] for _ in range(KD)]

            def t1w(sl, kd, c0, c1):
                return t1sh[sl][kd // 3][:, kd % 3, c0:c1]

            # head of the whole quant pipeline: hc0's kd0 column-block
            # alone first, so the first real matmul starts ~0.5us sooner
            quant_dve(w1t[0][:, 0:1, 0:128],
                      t1sh[0][0][:, 0:1, 0:128],
                      g1h, g1n, "qb1q", FP16, bbufs=2)
            quant_dve(w1t[0][:, 1:3, 0:128],
                      t1sh[0][0][:, 1:3, 0:128],
                      g1h, g1n, "qb1q", FP16, bbufs=2)
            quant_dve(w1t[0][:, 3:6, 0:128],
                      t1sh[0][1][:, :, 0:128],
                      g1h, g1n, "qb1q", FP16, bbufs=2)
            for tr in range(2):
                quant_dve(w1t[0][:, 3 * tr:3 * tr + 3, 128:256],
                          t1sh[0][tr][:, :, 128:256],
                          g1h, g1n, "qb1q", FP16, bbufs=2)
            s1_signs = [None, None]  # per half-slice, set in hooks
            if _NO_SIGN_ASSIST:
                for tr in range(2):
                    quant_dve(w1t[0][:, 3 * tr:3 * tr + 3, 256:512],
                              t1sh[0][tr][:, :, 256:512], g1h, g1n,
                              "qb1q2", FP16)
                for hf in range(2):
                    quant_dve(w1t[1][:, 3 * hf:3 * hf + 3, :],
                              t1sh[1][hf], g1h, g1n, "qb1s1", FP16)
            else:
                # ---- slice0 cols 256:512 + slice1: ACT-assisted ----
                # ACT: sign-pairs per kd-triple (before gelu hc0)
                s0b_signs = []
                for tr in range(2):
                    s0b_signs.append(quant_sign_act(
                        w1t[0][:, 3 * tr:3 * tr + 3, 256:512],
                        g1h, g1n, "sh"))
                # slice1 half 0's signs also ahead of gelu hc0
                s1_signs[0] = quant_sign_act(
                    w1t[1][:, 0:3, :], g1h, g1n, "sf")
                # DVE combine for s0B
                for tr in range(2):
                    sa, sb = s0b_signs[tr]
                    nc.vector.tensor_tensor(
                        out=t1sh[0][tr][:, :, 256:512],
                        in0=sa, in1=sb, op=Alu.add)

            # slice1 sign-pairs are interleaved with fc1(0) gelus; the
            # DVE combines + remaining quant jobs are emitted below via
            # deferred lists consumed by fc1(0)'s hc loop hooks.

            # ---- fc1 / fc2 ----
            hbt = {}

            def gelu_scale(hc):
                # slices quantized via ACT-assist hold {-2,0,2}
                if _NO_SIGN_ASSIST:
                    return g1f
                return g1h if 2 <= hc <= 7 else g1f

            def fc1(s, hooks=None):
                for hc in range(KH):
                    sl, off = hc // 4, (hc % 4) * 128
                    ps = psp.tile([128, 2, 512], FP32, tag="ps")
                    for kd in range(KD):
                        lhsT = t1w(sl, kd, off, off + 128)
                        for blk in range(2):
                            nc.tensor.matmul(
                                ps[:, blk, 0:TN], lhsT,
                                xs[s][:, kd, blk * TN:(blk + 1) * TN],
                                start=(kd == 0), stop=(kd == KD - 1))
                    hbv = hp.tile([128, 2, TN], BF16, tag="hb")
                    nc.scalar.activation(hbv, ps[:, :, 0:TN], Act.Gelu,
                                         bias=b1sb[:, hc:hc + 1],
                                         scale=gelu_scale(hc)[:, 0:1])
                    hbt.setdefault(s, []).append(hbv)
                    if hooks:
                        hooks(hc)

            def fc1_0_hooks(hc):
                # slice1 half 1's ACT sign-pair after gelu hc0
                if hc in (0, 1, 2):
                    if not _NO_SIGN_ASSIST and hc == 0:
                        s1_signs[1] = quant_sign_act(
                            w1t[1][:, 3:6, :], g1h, g1n, "sf")
                    if hc == 2:
                        # DVE combines for slice1 (chase the signs)
                        if not _NO_SIGN_ASSIST:
                            for hf in range(2):
                                sa, sb = s1_signs[hf]
                                nc.vector.tensor_tensor(
                                    out=t1sh[1][hf],
                                    in0=sa, in1=sb, op=Alu.add)
                        # slices 2-5: DVE2 half-slice jobs
                        last_q = None
                        for sl in range(2, NSL):
                            for hf in range(2):
                                last_q = quant_dve(
                                    w1t[sl][:, 3 * hf:3 * hf + 3, :],
                                    t1sh[sl][hf], g1h, g1n,
                                    "qb1", FP16)
                        # |w2| for b0 (lands last) on the then-idle DVE;
                        # forced after the w1 quant stream so its DMA
                        # wait can't head-of-line-block the quant
                        red_b0 = nc.vector.tensor_reduce(
                            out=acc2[:, 10:11], in_=w2t[0], axis=AxXY,
                            op=Alu.add, apply_absolute_value=True)
                        add_dep_helper(red_b0.ins, last_q.ins,
                                       reason="b0 abs after w1 quant")
                elif 8 <= hc <= 17:
                    # |w2| for b5..b1 on ACT, [128,2,768] half-jobs in
                    # landing order (same slot as rdump)
                    j = hc - 8
                    bt, half = 5 - j // 2, j % 2
                    rd2 = dp.tile([128, 2, D], BF16, tag="rdump", bufs=1,
                                  name="rd2")
                    nc.scalar.activation(
                        rd2, w2t[bt][:, 2 * half:2 * half + 2, :],
                        Act.Abs, accum_out=acc2[:, j:j + 1])
                if hc == 19:
                    # gamma2 + w2 quant: emitted only now, AFTER all
                    # acc2 writers exist (the dep tracker links a read
                    # only to writes emitted before it)
                    gam2 = gamma_chain(acc2, D * H, "g2")
                    g2_holder.append(gam2)
                    # both halves are consumed by fc2(0) (hf is the
                    # OUTPUT dc split); quant in landing order b5..b0,
                    # hf0 (needed first) before hf1
                    for hf2 in range(2):
                        for bt2 in range(KD - 1, -1, -1):
                            quant_dve(
                                w2t[bt2][:, :,
                                         hf2 * 384:(hf2 + 1) * 384],
                                t2bh[bt2][hf2], gam2[1], gam2[2],
                                "qb2", FP32)

            g2_holder = []

            # kh in w2-landing order (b5..b0): t2 quant for the first-
            # consumed batches is ready earliest
            kh_order = [kh for bt in range(KD - 1, -1, -1)
                        for kh in range(4 * bt, 4 * bt + 4)]

            def fc2(s, g2f_):
                for dc0, gsz in ((0, 3), (3, 2), (5, 1)):
                    pss = [psp.tile([128, 2, 512], FP32, tag="ps",
                                    name="ps") for _ in range(gsz)]
                    for ki, kh in enumerate(kh_order):
                        for i in range(gsz):
                            dc = dc0 + i
                            hf, col = (0, dc) if dc < 3 else (1, dc - 3)
                            lhsT = t2bh[kh // 4][hf][:, kh % 4,
                                                     col * 128:col * 128 + 128]
                            for blk in range(2):
                                nc.tensor.matmul(
                                    pss[i][:, blk, 0:TN], lhsT,
                                    hbt[s][kh][:, blk, :],
                                    start=(ki == 0), stop=(ki == KH - 1))
                    for i in range(gsz):
                        dc = dc0 + i
                        ysb = yp.tile([128, 2, TN], BF16, tag="ysb")
                        if s == 1 and gsz == 1:
                            # final group: split ACT/DVE for min tail
                            nc.scalar.activation(ysb[:, 0, :],
                                                 pss[i][:, 0, 0:TN],
                                                 Act.Identity,
                                                 bias=b2sb[:, dc:dc + 1],
                                                 scale=g2f_[:, 0:1])
                            nc.vector.tensor_scalar(
                                out=ysb[:, 1, :], in0=pss[i][:, 1, 0:TN],
                                scalar1=g2f_[:, 0:1],
                                scalar2=b2sb[:, dc:dc + 1],
                                op0=Alu.mult, op1=Alu.add)
                        else:
                            nc.scalar.activation(ysb, pss[i][:, :, 0:TN],
                                                 Act.Identity,
                                                 bias=b2sb[:, dc:dc + 1],
                                                 scale=g2f_[:, 0:1])
                        eng = nc.gpsimd if dc % 2 == 0 else nc.sync
                        eng.dma_start(
                            out=yt[dc * 128:(dc + 1) * 128,
                                   s * STN:(s + 1) * STN],
                            in_=ysb)
                del hbt[s]

            fc1(0, hooks=fc1_0_hooks)
            g2f = g2_holder[0][0]
            fc2(0, g2f)
            fc1(1)
            fc2(1, g2f)

    nc.compile()
    return nc


_NC = None


def _get_nc():
    global _NC
    if _NC is None:
        _NC = build()
    return _NC


def kernel(x, w1, b1, w2, b2, _trace=False, _trace_kwargs=None):
    nc = _get_nc()
    x = np.asarray(x, dtype=np.float32)
    w1 = np.asarray(w1, dtype=np.float32)
    b1 = np.asarray(b1, dtype=np.float32)
    w2 = np.asarray(w2, dtype=np.float32)
    b2 = np.asarray(b2, dtype=np.float32)
    x2 = np.ascontiguousarray(x.reshape(TOK, D).T).astype(ml_dtypes.bfloat16)
    # w1.T is [D, H]; wt1[s, p, k, c] = w1.T[k*128+p, s*512+c], fp16
    wt1 = np.ascontiguousarray(
        w1.T.astype(np.float16).reshape(KD, 128, NSL, W1S)
        .transpose(2, 1, 0, 3))
    wt2 = np.ascontiguousarray(w2.T)                    # [3072, 768]
    b1r = np.ascontiguousarray(b1.reshape(KH, 128).T)   # [128, 24]
    b2r = np.ascontiguousarray(b2.reshape(KD, 128).T)   # [128, 6]
    in_maps = []
    for c in range(N_CORES):
        xc = x2[:, c * TOK_PER:(c + 1) * TOK_PER]       # [768, 1576]
        xc = xc.reshape(KD, 128, NS, STN).transpose(1, 2, 0, 3)
        in_maps.append({
            "xt": np.ascontiguousarray(xc),
            "wt1": wt1, "wt2": wt2, "b1r": b1r, "b2r": b2r,
        })
    out = run_bass_kernel_spmd(nc, in_maps, list(range(N_CORES)),
                               trace=_trace, **(_trace_kwargs or {}))
    res = out.results
    yt = np.concatenate([res[c]["yt"].astype(np.float32) for c in
                         range(N_CORES)], axis=1)
    y = np.ascontiguousarray(yt.T).reshape(B, S, D)
    if _trace:
        return y, out
    return y
